# revision 21
# baseline (speedup 1.0000x reference)
"""Trainium2 Bass kernel for the BMP loss (nn_BMPLoss_24670292148307). V4b.

Data-parallel over 8 NeuronCores; host combines per-core partial sums.

V4b vs the 34us V3 baseline (critical-path + queue restructuring; vertex
math identical to V3: DVE add of pre-negated gt + ACT Abs-accumulate):
  - va/(-gt) ship interleaved per chunk in ONE dram tensor: one DMA
    trigger per chunk (DIRECT2D costs ~0.6us of sequencer time each),
    all on the SP HWDGE queue; blk/cst/out ride the ACT queue.  4 chunks
    sized [512,1792,1792,1536] -- the small first chunk starts the ACT
    Abs pipeline ~2us earlier and the add/abs of each chunk overlaps the
    DMA of the next.
  - Chebyshev outer-root evaluation in power basis: build [r..r^9] in 4
    DVE ops + one mul + one reduce (~0.5us serial) instead of the
    10-step Horner chain (2.1us serial).
  - A^2 computed right after A (off the critical path); det(K) branch
    fills the p=sqrt() ACT wait; the scale coefficient (pv2*v1i/3) is
    precomputed before the s=sqrt(lam) wait so only 4 small ops remain
    after it.
  - pose/betas/shape masks folded into the host pack (rows pre-zeroed);
    the mask multiplies and the comp memset are dropped (host sums only
    the rows each accumulator writes).
  - trailing probe op (comp col 7, ignored by host): tensor_scalar
    abs_max with accum_out on a full chunk, to read its perf mode from
    the trace.
"""
import numpy as np
from contextlib import ExitStack

import concourse.bass as bass
import concourse.bacc as bacc
import concourse.tile as tile
import concourse.mybir as mybir
from concourse.bass_utils import run_bass_kernel_spmd

f32 = mybir.dt.float32
bf16 = mybir.dt.bfloat16
AF = mybir.ActivationFunctionType
OP = mybir.AluOpType
AX = mybir.AxisListType

B_PER_CORE = 64
N_CORES = 8
J = 24
VERT_F = 20670           # real floats per sample (6890*3)
F_PACK = 5632            # padded stream cols per tensor
PACK_CAP = 34            # vertex slots per core (128*5632/20670 = 34.8)
W_CHUNKS = [1792, 1792, 1536, 512]
EPS = 1e-8
TINY = 1e-30
RCLAMP = 0.99995

# blk_a (bf16): procrustes-critical columns, lands first
PG6 = slice(0, 144)      # (c,n): rows 0-2 pj xyz, 3-5 gt3 xyz, joint-minor
CONF3 = slice(144, 168)
CAM = slice(168, 171)
BLKA_COLS = 171
# blk_b (bf16): the rest
G2 = slice(0, 48)        # (c,n), pre-shifted by -256, /1000
CONF2 = slice(48, 72)
RP = slice(72, 288)      # pre-masked by has_smpl
RG = slice(288, 504)     # pre-masked
PB = slice(504, 514)     # pre-masked
GS = slice(514, 524)     # pre-masked
BLKB_COLS = 524

# cst (f32) column map
PC = slice(0, 18)        # cols 0-8 = P1C[1..9], 9-17 = P3C[1..9]
C0 = slice(18, 20)       # [P1C[0], P3C[0]]
EYE9 = slice(20, 29)
EYE3 = slice(29, 38)     # eye/3
RCL = slice(38, 39)      # RCLAMP
TNY = slice(39, 40)      # TINY
CST_COLS = 40

P1C = [0.8649274597522203, 0.17578197434414333, -0.002087134697444787,
       -0.1271791091353304, -0.3070988770461487, 0.6789215326112841,
       0.5727490378285598, -1.068537975408937, -0.3683220235409602,
       0.5818562170395759]
P3C = [-0.8649274597522203, 0.17578197434414353, 0.002087134697442622,
       -0.1271791091353331, 0.3070988770461617, 0.6789215326112932,
       -0.5727490378285826, -1.068537975408948, 0.3683220235409723,
       0.58185621703958]


def _cst_array() -> np.ndarray:
    c = np.zeros((B_PER_CORE, CST_COLS), np.float32)
    for t in range(9):
        c[:, t] = np.float32(P1C[t + 1])
        c[:, 9 + t] = np.float32(P3C[t + 1])
    c[:, 18] = np.float32(P1C[0])
    c[:, 19] = np.float32(P3C[0])
    eye = np.eye(3, dtype=np.float32).reshape(9)
    c[:, EYE9] = eye
    c[:, EYE3] = eye / 3.0
    c[:, RCL] = RCLAMP
    c[:, TNY] = TINY
    return c


def build_program():
    nc = bacc.Bacc("TRN2", target_bir_lowering=False, debug=False,
                   num_devices=N_CORES)
    P = B_PER_CORE

    cst_d = nc.dram_tensor("cst", [P, CST_COLS], f32, kind="ExternalInput")
    blka_d = nc.dram_tensor("blka", [P, BLKA_COLS], bf16,
                            kind="ExternalInput")
    blkb_d = nc.dram_tensor("blkb", [P, BLKB_COLS], bf16,
                            kind="ExternalInput")
    vab_d = nc.dram_tensor("vab", [128, 2 * F_PACK], bf16,
                           kind="ExternalInput")
    out_d = nc.dram_tensor("out", [128, 8], f32, kind="ExternalOutput")

    with tile.TileContext(nc) as tc, ExitStack() as ctx:
        V = nc.vector
        A = nc.scalar
        G = nc.gpsimd
        SP = nc.sync
        sg = ctx.enter_context(tc.tile_pool(name="singles", bufs=1))
        vp = ctx.enter_context(tc.tile_pool(name="vp", bufs=2))

        def S(shape, name, dtype=f32):
            return sg.tile(list(shape), dtype, name=name)

        comp = S([128, 8], "comp")

        # first ACT op is a Sqrt so the table loader picks the sqrt set once
        warm = S([1, 1], "warm")
        G.memset(warm[:, :], 1.0)
        warm2 = S([1, 1], "warm2")
        A.activation(warm2[:, :], warm[:, :], AF.Sqrt)

        # ---------------- input DMAs ----------------------------------------
        blk_t = S([P, BLKA_COLS], "blk_t", bf16)
        A.dma_start(blk_t[:, :], blka_d[:, :])
        cst_t = S([P, CST_COLS], "cst_t")
        A.dma_start(cst_t[:, :], cst_d[:, :])
        blkb_t = S([P, BLKB_COLS], "blkb_t", bf16)
        A.dma_start(blkb_t[:, :], blkb_d[:, :])
        vab_ts = []
        off = 0
        for c, w in enumerate(W_CHUNKS):
            vt = sg.tile([128, 2 * w], bf16, name=f"vab{c}")
            SP.dma_start(vt[:, :], vab_d[:, 2 * off:2 * off + 2 * w])
            vab_ts.append(vt)
            off += w

        pg6 = blk_t[:, PG6]
        eye9 = cst_t[:, EYE9]
        eye3 = cst_t[:, EYE3]
        t1 = S([P, 1], "t1")

        # ================ Procrustes chain (DVE) ================
        musum = S([P, 6], "musum")
        V.tensor_reduce(musum[:, :], pg6.rearrange("p (c n) -> p c n", n=J),
                        axis=AX.X, op=OP.add)
        Xn = S([P, 144], "Xn", bf16)   # (musum/24 - pg6): negated centered
        V.scalar_tensor_tensor(
            Xn[:, :].rearrange("p (c n) -> p c n", n=J),
            musum[:, :].unsqueeze(2).broadcast_to([P, 6, J]), 1.0 / J,
            pg6.rearrange("p (c n) -> p c n", n=J), OP.mult, OP.subtract)
        X1n = Xn[:, 0:72]
        X2n = Xn[:, 72:144]
        var1 = S([P, 1], "var1")
        vscr = S([P, 72], "vscr")
        A.activation(vscr[:, :], X1n, AF.Square, accum_out=var1[:, :])
        G.tensor_scalar(t1[:, :], blk_t[:, CAM][:, 0:1], 512.0, EPS,
                        OP.mult, OP.add)
        rt1 = S([P, 1], "rt1")
        V.reciprocal(rt1[:, :], t1[:, :])

        # kp2d prep front-loaded on Pool so rzt slots into the chain early
        depth = S([P, 1], "depth")
        G.tensor_single_scalar(depth[:, :], rt1[:, :], 2000.0, OP.mult)
        pxy = S([P, 48], "pxy", bf16)
        G.tensor_add(pxy[:, :].rearrange("p (c n) -> p c n", n=J),
                     blk_t[:, PG6].rearrange("p (c n) -> p c n", n=J)[:, 0:2],
                     blk_t[:, CAM][:, 1:3].unsqueeze(2).broadcast_to([P, 2, J]))
        pzt = S([P, J], "pzt")
        G.tensor_add(pzt[:, :], blk_t[:, 48:72],
                     depth[:, :].broadcast_to([P, J]))

        # K = X1 X2^T
        kq = S([P, 216], "kq", bf16)
        V.tensor_mul(
            kq[:, :].rearrange("p (i j n) -> p i j n", i=3, j=3),
            X1n.rearrange("p (i n) -> p i n", i=3)
                .unsqueeze(2).broadcast_to([P, 3, 3, J]),
            X2n.rearrange("p (j n) -> p j n", j=3)
                .unsqueeze(1).broadcast_to([P, 3, 3, J]))
        K9 = S([P, 9], "K9")
        i_K9 = V.tensor_reduce(K9[:, :], kq[:, :].rearrange(
            "p (i j n) -> p i j n", i=3, j=3), axis=AX.X, op=OP.add)

        # det(K) on DVE (feeds detA for r, and the sign)
        dQ = S([P, 9], "dQ")
        V.tensor_mul(
            dQ[:, :].rearrange("p (a b) -> p a b", a=3),
            K9[:, 3:6].unsqueeze(2).broadcast_to([P, 3, 3]),
            K9[:, 6:9].unsqueeze(1).broadcast_to([P, 3, 3]))
        dD = S([P, 9], "dD")
        V.tensor_sub(dD[:, :].rearrange("p (a b) -> p a b", a=3),
                     dQ[:, :].rearrange("p (a b) -> p a b", a=3),
                     dQ[:, :].rearrange("p (b a) -> p a b", b=3))
        du1 = S([P, 2], "du1")
        i_du1 = G.tensor_mul(du1[:, :], K9[:, 0:2], dD[:, 5:7])
        du2 = S([P, 1], "du2")
        G.tensor_mul(du2[:, :], K9[:, 2:3], dD[:, 1:2])
        du1r = S([P, 1], "du1r")
        G.tensor_add(du1r[:, :], du1[:, 0:1], du1[:, 1:2])
        detK = S([P, 1], "detK")
        G.tensor_add(detK[:, :], du1r[:, :], du2[:, :])
        detA = S([P, 1], "detA")
        G.tensor_mul(detA[:, :], detK[:, :], detK[:, :])
        sg0 = S([P, 1], "sg0")
        G.tensor_single_scalar(sg0[:, :], detK[:, :], 0.0, OP.is_ge)
        sgn = S([P, 1], "sgn")
        i_sgn = G.tensor_scalar(sgn[:, :], sg0[:, :], 2.0, -1.0,
                                OP.mult, OP.add)

        # A = K^T K
        aq = S([P, 27], "aq")  # keep f32: A feeds the eigen chain
        V.tensor_mul(
            aq[:, :].rearrange("p (i j k) -> p i j k", i=3, j=3),
            K9[:, :].rearrange("p (k i) -> p i k", k=3)
                .unsqueeze(2).broadcast_to([P, 3, 3, 3]),
            K9[:, :].rearrange("p (k j) -> p j k", k=3)
                .unsqueeze(1).broadcast_to([P, 3, 3, 3]))
        A9 = S([P, 9], "A9")
        V.tensor_reduce(A9[:, :], aq[:, :].rearrange(
            "p (i j k) -> p i j k", i=3, j=3), axis=AX.X, op=OP.add)
        # A^2 on Pool (mul + 2 strided adds): needed only at W
        a2q = S([P, 27], "a2q")
        G.tensor_mul(
            a2q[:, :].rearrange("p (i j k) -> p i j k", i=3, j=3),
            A9[:, :].rearrange("p (i k) -> p i k", i=3)
                .unsqueeze(2).broadcast_to([P, 3, 3, 3]),
            A9[:, :].rearrange("p (k j) -> p j k", k=3)
                .unsqueeze(1).broadcast_to([P, 3, 3, 3]))
        a2p = S([P, 9], "a2p")
        G.tensor_add(a2p[:, :],
                     a2q[:, :].rearrange("p (m k) -> p m k", k=3)[:, :, 0],
                     a2q[:, :].rearrange("p (m k) -> p m k", k=3)[:, :, 1])
        A29 = S([P, 9], "A29")
        G.tensor_add(A29[:, :], a2p[:, :],
                     a2q[:, :].rearrange("p (m k) -> p m k", k=3)[:, :, 2])

        qsum = S([P, 1], "qsum")
        V.tensor_reduce(qsum[:, :], A9[:, 0:9:4], axis=AX.X, op=OP.add)
        q3rd = S([P, 1], "q3rd")
        V.tensor_single_scalar(q3rd[:, :], qsum[:, :], 1.0 / 3.0, OP.mult)
        q2 = S([P, 1], "q2")
        V.tensor_mul(q2[:, :], q3rd[:, :], q3rd[:, :])
        nqsum = S([P, 1], "nqsum")
        V.tensor_single_scalar(nqsum[:, :], qsum[:, :], -1.0, OP.mult)
        aqn = S([P, 9], "aqn")
        V.scalar_tensor_tensor(aqn[:, :], eye3, qsum[:, :], A9[:, :],
                               OP.mult, OP.subtract)
        pscr = S([P, 9], "pscr")
        V.tensor_mul(pscr[:, :], aqn[:, :], aqn[:, :])
        p2r = S([P, 1], "p2r")
        V.tensor_reduce(p2r[:, :], pscr[:, :], axis=AX.X, op=OP.add)
        p2g = S([P, 1], "p2g")
        i_p2g = V.tensor_scalar(p2g[:, :], p2r[:, :], 1.0 / 6.0, TINY,
                                OP.mult, OP.max)
        pp = S([P, 1], "pp")
        i_pp = A.activation(pp[:, :], p2g[:, :], AF.Sqrt)
        tp = S([P, 1], "tp")
        V.tensor_single_scalar(tp[:, :], pp[:, :], 2.0, OP.mult)

        # z = detA + q*(3 p^2 - q^2)  (fills the sqrt wait)
        zq = S([P, 1], "zq")
        V.scalar_tensor_tensor(zq[:, :], p2g[:, :], 3.0, q2[:, :],
                               OP.mult, OP.subtract)
        zv = S([P, 1], "zv")
        V.tensor_mul(zv[:, :], q3rd[:, :], zq[:, :])
        zz = S([P, 1], "zz")
        V.tensor_add(zz[:, :], detA[:, :], zv[:, :])

        # ---------------- kp3d (Pool prep, ACT accumulate) ------------------
        pd = S([P, 72], "pd", bf16)
        i_pd = G.tensor_sub(pd[:, :], blk_t[:, 0:72], blk_t[:, 72:144])
        pdr = pd[:, :].rearrange("p (c n) -> p c n", n=J)
        pel = S([P, 3], "pel", bf16)
        G.tensor_add(pel[:, :], pdr[:, :, 2].squeeze(), pdr[:, :, 3].squeeze())
        pel2 = S([P, 3], "pel2", bf16)
        G.tensor_single_scalar(pel2[:, :], pel[:, :], 0.5, OP.mult)
        d3n = S([P, 72], "d3n", bf16)
        G.tensor_sub(d3n[:, :].rearrange("p (c n) -> p c n", n=J),
                     pdr, pel2[:, :].unsqueeze(2).broadcast_to([P, 3, J]))
        u3d = S([P, 72], "u3d", bf16)
        G.tensor_mul(u3d[:, :].rearrange("p (c n) -> p c n", n=J),
                     d3n[:, :].rearrange("p (c n) -> p c n", n=J),
                     blk_t[:, CONF3].unsqueeze(1).broadcast_to([P, 3, J]))
        kscr3 = S([P, 72], "kscr3")
        A.activation(kscr3[:, :], u3d[:, :], AF.Abs,
                     accum_out=comp[0:P, 1:2])

        # pose/betas subs (Pool) + Square-accumulate (ACT); host pre-masked
        dp = S([P, 216], "dp", bf16)
        i_dp = G.tensor_sub(dp[:, :], blkb_t[:, RP], blkb_t[:, RG])
        pscr2 = S([P, 216], "pscr2", bf16)
        i_pscr2 = A.activation(pscr2[:, :], dp[:, :], AF.Square,
                               accum_out=comp[0:P, 3:4])
        db = S([P, 10], "db", bf16)
        G.tensor_sub(db[:, :], blkb_t[:, PB], blkb_t[:, GS])
        bscr = S([P, 10], "bscr", bf16)
        A.activation(bscr[:, :], db[:, :], AF.Square,
                     accum_out=comp[0:P, 4:5])

        # r = clamp(z/(2 p p^2)) via a single reciprocal
        up = S([P, 1], "up")
        V.tensor_mul(up[:, :], tp[:, :], p2g[:, :])
        ru = S([P, 1], "ru")
        V.reciprocal(ru[:, :], up[:, :])
        r1 = S([P, 1], "r1")
        i_r1 = V.scalar_tensor_tensor(r1[:, :], zz[:, :], ru[:, :],
                                      cst_t[:, RCL], OP.mult, OP.min)
        # pinv/pv2 feed only the scale coefficient: off the critical path
        pinv = S([P, 1], "pinv")
        i_pinv = V.reciprocal(pinv[:, :], pp[:, :])
        pv2 = S([P, 1], "pv2")
        V.tensor_mul(pv2[:, :], pinv[:, :], pinv[:, :])

        # powers of r: pw = [r, r^2, ..., r^9]
        pw = S([P, 9], "pw")
        V.tensor_single_scalar(pw[:, 0:1], r1[:, :], -RCLAMP, OP.max)
        V.tensor_mul(pw[:, 1:2], pw[:, 0:1], pw[:, 0:1])
        V.tensor_scalar_mul(pw[:, 2:4], pw[:, 0:2], pw[:, 1:2])
        V.tensor_scalar_mul(pw[:, 4:8], pw[:, 0:4], pw[:, 3:4])
        V.tensor_mul(pw[:, 8:9], pw[:, 3:4], pw[:, 4:5])
        # both outer-root polynomials from one mul + one reduce
        pprod = S([P, 18], "pprod")
        V.tensor_mul(pprod[:, :].rearrange("p (g t) -> p g t", g=2),
                     cst_t[:, PC].rearrange("p (g t) -> p g t", g=2),
                     pw[:, :].unsqueeze(1).broadcast_to([P, 2, 9]))
        xr = S([P, 2], "xr")
        V.tensor_reduce(xr[:, :], pprod[:, :].rearrange(
            "p (g t) -> p g t", g=2), axis=AX.X, op=OP.add)
        x = S([P, 2], "xroots")
        V.tensor_add(x[:, :], xr[:, :], cst_t[:, C0])

        # rzt here: Pool's pzt is ready by now, so DVE never stalls on it
        rzt = S([P, J], "rzt")
        i_rzt = V.reciprocal(rzt[:, :], pzt[:, :])
        aa = S([P, 48], "aa")
        G.tensor_mul(aa[:, :].rearrange("p (c n) -> p c n", n=J),
                     pxy[:, :].rearrange("p (c n) -> p c n", n=J),
                     rzt[:, :].unsqueeze(1).broadcast_to([P, 2, J]))
        dkp = S([P, 48], "dkp")
        G.tensor_sub(dkp[:, :], aa[:, :], blkb_t[:, G2])
        u2d = S([P, 48], "u2d")
        G.tensor_mul(u2d[:, :].rearrange("p (c n) -> p c n", n=J),
                     dkp[:, :].rearrange("p (c n) -> p c n", n=J),
                     blkb_t[:, CONF2].unsqueeze(1).broadcast_to([P, 2, J]))
        kscr = S([P, 48], "kscr")
        A.activation(kscr[:, :], u2d[:, :], AF.Abs,
                     accum_out=comp[0:P, 0:1])

        # eigenvalues: lam = [l1, lmid, l3=detA/(l1*lmid)], clamped >= TINY
        lamt = S([P, 3], "lamt")
        V.scalar_tensor_tensor(lamt[:, 0:3:2], x[:, :], tp[:, :],
                               q3rd[:, :].broadcast_to([P, 2]),
                               OP.mult, OP.add)
        t13 = S([P, 1], "t13")
        V.tensor_add(t13[:, :], lamt[:, 0:1], lamt[:, 2:3])
        V.tensor_sub(lamt[:, 1:2], qsum[:, :], t13[:, :])
        t12g = S([P, 1], "t12g")
        V.scalar_tensor_tensor(t12g[:, :], lamt[:, 0:1], lamt[:, 1:2],
                               cst_t[:, TNY], OP.mult, OP.max)
        rt12 = S([P, 1], "rt12")
        V.reciprocal(rt12[:, :], t12g[:, :])
        V.tensor_mul(lamt[:, 2:3], detA[:, :], rt12[:, :])
        lam = S([P, 3], "lam")
        i_lam = V.tensor_single_scalar(lam[:, :], lamt[:, :], TINY, OP.max)
        s3t = S([P, 3], "s3t")
        i_s3t = A.activation(s3t[:, :], lam[:, :], AF.Sqrt)

        # fill the sqrt wait: gap products + scale coefficient
        v1i = S([P, 1], "v1i")
        V.reciprocal(v1i[:, :], var1[:, :])
        cpre = S([P, 1], "cpre")   # pv2 * v1i / 3
        V.scalar_tensor_tensor(cpre[:, :], pv2[:, :], 1.0 / 3.0,
                               v1i[:, :], OP.mult, OP.mult)
        gA = S([P, 2], "gA")   # [l1-lmid, lmid-l3]
        V.tensor_sub(gA[:, :], lam[:, 0:2], lam[:, 1:3])
        g02 = S([P, 1], "g02")
        V.tensor_add(g02[:, :], gA[:, 0:1], gA[:, 1:2])
        Dt = S([P, 3], "Dt")   # signed gap products
        V.tensor_mul(Dt[:, 0:1], gA[:, 0:1], g02[:, :])
        V.scalar_tensor_tensor(Dt[:, 1:2], gA[:, 0:1], -1.0, gA[:, 1:2],
                               OP.mult, OP.mult)
        V.scalar_tensor_tensor(Dt[:, 2:3], g02[:, :], sgn[:, :],
                               gA[:, 1:2], OP.mult, OP.mult)
        rD = S([P, 3], "rD")
        V.reciprocal(rD[:, :], Dt[:, :])

        # scl = (s1+s2+sgn*s3) * cpre  (post-sqrt: 4 small ops)
        sinv = S([P, 3], "sinv")
        V.reciprocal(sinv[:, :], s3t[:, :])
        s0s2 = S([P, 1], "s0s2")
        V.scalar_tensor_tensor(s0s2[:, :], s3t[:, 2:3], sgn[:, :],
                               s3t[:, 0:1], OP.mult, OP.add)
        ssum = S([P, 1], "ssum")
        V.tensor_add(ssum[:, :], s0s2[:, :], s3t[:, 1:2])
        scl = S([P, 1], "scl")
        V.tensor_mul(scl[:, :], ssum[:, :], cpre[:, :])

        # mm9: [m | m*lam | m*linv] -> one reduce gives (al2, t1, t0)
        linv = S([P, 3], "linv")
        V.tensor_mul(linv[:, :], sinv[:, :], sinv[:, :])
        mm9 = S([P, 9], "mm9")
        V.tensor_mul(mm9[:, 0:3], rD[:, :], sinv[:, :])
        V.tensor_mul(mm9[:, 3:6], mm9[:, 0:3], lam[:, :])
        V.tensor_mul(mm9[:, 6:9], mm9[:, 0:3], linv[:, :])
        asum = S([P, 3], "asum")
        i_asum = V.tensor_reduce(asum[:, :], mm9[:, :].rearrange(
            "p (g i) -> p g i", g=3), axis=AX.X, op=OP.add)
        al1 = S([P, 1], "al1")
        V.scalar_tensor_tensor(al1[:, :], asum[:, 0:1], nqsum[:, :],
                               asum[:, 1:2], OP.mult, OP.add)
        al0 = S([P, 1], "al0")
        V.tensor_mul(al0[:, :], asum[:, 2:3], detA[:, :])

        aI = S([P, 9], "aI")
        V.tensor_scalar_mul(aI[:, :], eye9, al0[:, :])
        W1 = S([P, 9], "W1")
        V.scalar_tensor_tensor(W1[:, :], A29[:, :], asum[:, 0:1], aI[:, :],
                               OP.mult, OP.add)
        W9 = S([P, 9], "W9")
        V.scalar_tensor_tensor(W9[:, :], A9[:, :], al1[:, :], W1[:, :],
                               OP.mult, OP.add)

        # R = W K^T ; RX1 ; Y ; d2
        rq = S([P, 27], "rq")
        V.tensor_mul(
            rq[:, :].rearrange("p (a b c) -> p a b c", a=3, b=3),
            W9[:, :].rearrange("p (a c) -> p a c", a=3)
                .unsqueeze(2).broadcast_to([P, 3, 3, 3]),
            K9[:, :].rearrange("p (b c) -> p b c", b=3)
                .unsqueeze(1).broadcast_to([P, 3, 3, 3]))
        R9b = S([P, 9], "R9b", bf16)
        with nc.allow_low_precision(reason="R entries; 3-term reduce"):
            V.tensor_reduce(R9b[:, :], rq[:, :].rearrange(
                "p (a b c) -> p a b c", a=3, b=3), axis=AX.X, op=OP.add)
        rxq = S([P, 216], "rxq", bf16)
        V.tensor_mul(
            rxq[:, :].rearrange("p (i n j) -> p i n j", i=3, n=J),
            R9b[:, :].rearrange("p (i j) -> p i j", i=3)
                .unsqueeze(2).broadcast_to([P, 3, J, 3]),
            X1n.rearrange("p (j n) -> p n j", j=3)
                .unsqueeze(1).broadcast_to([P, 3, J, 3]))
        rx1 = S([P, 72], "rx1")
        V.tensor_reduce(rx1[:, :].rearrange("p (i n) -> p i n", i=3),
                        rxq[:, :].rearrange("p (i n j) -> p i n j",
                                            i=3, n=J),
                        axis=AX.X, op=OP.add)
        Yt = S([P, 72], "Yt")
        V.scalar_tensor_tensor(Yt[:, :], rx1[:, :], scl[:, :], X2n,
                               OP.mult, OP.subtract)
        Y2 = S([P, 72], "Y2")
        V.tensor_mul(Y2[:, :], Yt[:, :], Yt[:, :])
        d2 = S([P, J], "d2")
        V.tensor_reduce(d2[:, :],
                        Y2[:, :].rearrange("p (i n) -> p n i", i=3),
                        axis=AX.X, op=OP.add)
        dscr = S([P, J], "dscr")
        A.activation(dscr[:, :], d2[:, :], AF.Sqrt,
                     accum_out=comp[0:P, 5:6])

        # ---------------- vertex L1 (DVE add + ACT Abs-accumulate) ----------
        vacc = S([128, len(W_CHUNKS)], "vacc")
        i_adds, i_abss = [], []
        for c, w in enumerate(W_CHUNKS):
            vt = vab_ts[c]
            d_t = vp.tile([128, w], bf16, name=f"d{c}", tag="d")
            i_adds.append(V.tensor_add(d_t[:, :], vt[:, 0:w],
                                       vt[:, w:2 * w]))
            s_t = vp.tile([128, w], bf16, name=f"s{c}", tag="s")
            i_abss.append(A.activation(s_t[:, :], d_t[:, :], AF.Abs,
                                       accum_out=vacc[:, c:c + 1]))
        V.tensor_reduce(comp[:, 2:3], vacc[:, :], axis=AX.X, op=OP.add)

        # schedule pins (add_dep_helper(a, b) == a waits on b): keep the
        # vertex adds in the chain's ACT-wait windows, the sqrts ahead of
        # the long Abs ops on ACT, and the Pool det-branch ahead of preps
        for dep, on, why in [
            (i_adds[0], i_K9, "add0 after the K front"),
            (i_adds[1], i_r1, "add1 after the r clamp"),
            (i_adds[2], i_lam, "add2 fills the s-sqrt wait"),
            (i_adds[3], i_asum, "add3 late in the chain"),
            (i_rzt, i_p2g, "rzt off the pre-sqrt region"),
            (i_abss[1], i_s3t, "both sqrts before the long abs1"),
            (i_abss[2], i_s3t, "s sqrt before the long abs2"),
            (i_pd, i_sgn, "Pool det-branch before kp3d prep"),
            (i_pscr2, i_pp, "p-sqrt before the pose square on ACT"),
            (i_pinv, i_r1, "scale-coefficient recip off the r path"),
            (i_dp, i_sgn, "Pool det-branch before pose prep"),
        ]:
            tile.add_dep_helper(dep.ins, on.ins, sync=False, reason=why)

        # ---------------- output (SP queue, split) --------------------------
        SP.dma_start(out_d[:, 0:5], comp[:, 0:5])
        SP.dma_start(out_d[:, 5:8], comp[:, 5:8])

    nc.compile()
    return nc


_PROGRAM = None


def _get_program():
    global _PROGRAM
    if _PROGRAM is None:
        _PROGRAM = build_program()
    return _PROGRAM


def make_in_maps(inputs: dict) -> list:
    import ml_dtypes

    pj = np.asarray(inputs["pred_joints"], np.float32)
    cam = np.asarray(inputs["pred_camera"], np.float32)
    g2 = np.asarray(inputs["gt_keypoints_2d"], np.float32)
    g3 = np.asarray(inputs["gt_keypoints_3d"], np.float32)
    rp = np.asarray(inputs["pred_rotmat"], np.float32).reshape(512, 216)
    rg = np.asarray(inputs["gt_rotmat"], np.float32).reshape(512, 216)
    pb = np.asarray(inputs["pred_betas"], np.float32)
    gs = np.asarray(inputs["gt_shape"], np.float32)
    hs = np.asarray(inputs["has_smpl"], np.int32)
    va = np.asarray(inputs["pred_vertices"], np.float32).reshape(512, VERT_F)
    vb = np.asarray(inputs["gt_vertices"], np.float32).reshape(512, VERT_F)
    cst = _cst_array()

    idx = np.nonzero(hs > 0)[0]
    assert idx.size <= N_CORES * PACK_CAP, (
        f"n_valid={idx.size} exceeds vertex pack capacity")

    def packed(src, sel, negate):
        buf = np.zeros(128 * F_PACK, ml_dtypes.bfloat16)
        if sel.size:
            flat = src[sel].reshape(-1)
            if negate:
                flat = -flat
            buf[:flat.size] = flat.astype(ml_dtypes.bfloat16)
        return buf.reshape(128, F_PACK)

    in_maps = []
    for c in range(N_CORES):
        sl = slice(B_PER_CORE * c, B_PER_CORE * (c + 1))
        sel = idx[c::N_CORES]
        mask = (hs[sl] > 0).astype(np.float32)[:, None]
        blka = np.empty((B_PER_CORE, BLKA_COLS), np.float32)
        blka[:, 0:72] = pj[sl].transpose(0, 2, 1).reshape(B_PER_CORE, 72)
        blka[:, 72:144] = g3[sl, :, :3].transpose(0, 2, 1).reshape(
            B_PER_CORE, 72)
        blka[:, CONF3] = g3[sl, :, 3]
        blka[:, CAM] = cam[sl]
        blkb = np.empty((B_PER_CORE, BLKB_COLS), np.float32)
        blkb[:, G2] = ((g2[sl, :, :2] - 256.0) / 1000.0).transpose(
            0, 2, 1).reshape(B_PER_CORE, 48)
        blkb[:, CONF2] = g2[sl, :, 2] * 1000.0
        blkb[:, RP] = rp[sl] * mask
        blkb[:, RG] = rg[sl] * mask
        blkb[:, PB] = pb[sl] * mask
        blkb[:, GS] = gs[sl] * mask
        va_p = packed(va, sel, False)
        vb_p = packed(vb, sel, True)
        vab = np.empty((128, 2 * F_PACK), ml_dtypes.bfloat16)
        off = 0
        for w in W_CHUNKS:
            vab[:, 2 * off:2 * off + w] = va_p[:, off:off + w]
            vab[:, 2 * off + w:2 * off + 2 * w] = vb_p[:, off:off + w]
            off += w
        in_maps.append({
            "cst": np.ascontiguousarray(cst, np.float32),
            "blka": np.ascontiguousarray(blka.astype(ml_dtypes.bfloat16)),
            "blkb": np.ascontiguousarray(blkb.astype(ml_dtypes.bfloat16)),
            "vab": np.ascontiguousarray(vab),
        })
    return in_maps


def combine_partials(parts: np.ndarray, n_valid: float) -> np.float32:
    # parts: [n_cores, 128, 8]
    p64 = parts.astype(np.float64)
    kp2d = p64[:, 0:B_PER_CORE, 0].sum()
    kp3d = p64[:, 0:B_PER_CORE, 1].sum()
    vert = p64[:, :, 2].sum()
    pose = p64[:, 0:B_PER_CORE, 3].sum()
    betas = p64[:, 0:B_PER_CORE, 4].sum()
    pa = p64[:, 0:B_PER_CORE, 5].sum()
    B = 512.0
    total = (4.0 * kp2d / (512.0 * B * J * 2)
             + 4.0 * kp3d / (B * J * 3)
             + vert / (n_valid * VERT_F + EPS)
             + pose / (n_valid * 216 + EPS)
             + 0.01 * betas / (n_valid * 10 + EPS)
             + pa / (B * J))
    return np.float32(total)


def kernel(**inputs) -> np.ndarray:
    nc = _get_program()
    in_maps = make_in_maps(inputs)
    res = run_bass_kernel_spmd(nc, in_maps, core_ids=list(range(N_CORES)))
    parts = np.stack([res.results[c]["out"] for c in range(N_CORES)])
    nv = float((np.asarray(inputs["has_smpl"]) > 0).sum())
    return np.asarray(combine_partials(parts, nv))


# revision 22
# speedup vs baseline: 1.0355x; 1.0355x over previous
"""Trainium2 Bass kernel for the BMP loss (nn_BMPLoss_24670292148307). V4b.

Data-parallel over 8 NeuronCores; host combines per-core partial sums.

V4b vs the 34us V3 baseline (critical-path + queue restructuring; vertex
math identical to V3: DVE add of pre-negated gt + ACT Abs-accumulate):
  - va/(-gt) ship interleaved per chunk in ONE dram tensor: one DMA
    trigger per chunk (DIRECT2D costs ~0.6us of sequencer time each),
    all on the SP HWDGE queue; blk/cst/out ride the ACT queue.  4 chunks
    sized [512,1792,1792,1536] -- the small first chunk starts the ACT
    Abs pipeline ~2us earlier and the add/abs of each chunk overlaps the
    DMA of the next.
  - Chebyshev outer-root evaluation in power basis: build [r..r^9] in 4
    DVE ops + one mul + one reduce (~0.5us serial) instead of the
    10-step Horner chain (2.1us serial).
  - A^2 computed right after A (off the critical path); det(K) branch
    fills the p=sqrt() ACT wait; the scale coefficient (pv2*v1i/3) is
    precomputed before the s=sqrt(lam) wait so only 4 small ops remain
    after it.
  - pose/betas/shape masks folded into the host pack (rows pre-zeroed);
    the mask multiplies and the comp memset are dropped (host sums only
    the rows each accumulator writes).
  - trailing probe op (comp col 7, ignored by host): tensor_scalar
    abs_max with accum_out on a full chunk, to read its perf mode from
    the trace.
"""
import numpy as np
from contextlib import ExitStack

import concourse.bass as bass
import concourse.bacc as bacc
import concourse.tile as tile
import concourse.mybir as mybir
from concourse.bass_utils import run_bass_kernel_spmd

f32 = mybir.dt.float32
bf16 = mybir.dt.bfloat16
AF = mybir.ActivationFunctionType
OP = mybir.AluOpType
AX = mybir.AxisListType

B_PER_CORE = 64
N_CORES = 8
J = 24
VERT_F = 20670           # real floats per sample (6890*3)
F_PACK = 5632            # padded stream cols per tensor
PACK_CAP = 34            # vertex slots per core (128*5632/20670 = 34.8)
W_CHUNKS = [1792, 1792, 1536, 512]
EPS = 1e-8
TINY = 1e-30
RCLAMP = 0.99995

# blk_a (bf16): procrustes-critical columns, lands first
PG6 = slice(0, 144)      # (c,n): rows 0-2 pj xyz, 3-5 gt3 xyz, joint-minor
CONF3 = slice(144, 168)
CAM = slice(168, 171)
BLKA_COLS = 171
# blk_b (bf16): the rest
G2 = slice(0, 48)        # (c,n), pre-shifted by -256, /1000
CONF2 = slice(48, 72)
RP = slice(72, 288)      # pre-masked by has_smpl
RG = slice(288, 504)     # pre-masked
PB = slice(504, 514)     # pre-masked
GS = slice(514, 524)     # pre-masked
BLKB_COLS = 524

# cst (f32) column map
PC = slice(0, 18)        # cols 0-8 = P1C[1..9], 9-17 = P3C[1..9]
C0 = slice(18, 20)       # [P1C[0], P3C[0]]
EYE9 = slice(20, 29)
EYE3 = slice(29, 38)     # eye/3
RCL = slice(38, 39)      # RCLAMP
TNY = slice(39, 40)      # TINY
CST_COLS = 40

P1C = [0.8649274597522203, 0.17578197434414333, -0.002087134697444787,
       -0.1271791091353304, -0.3070988770461487, 0.6789215326112841,
       0.5727490378285598, -1.068537975408937, -0.3683220235409602,
       0.5818562170395759]
P3C = [-0.8649274597522203, 0.17578197434414353, 0.002087134697442622,
       -0.1271791091353331, 0.3070988770461617, 0.6789215326112932,
       -0.5727490378285826, -1.068537975408948, 0.3683220235409723,
       0.58185621703958]


def _cst_array() -> np.ndarray:
    c = np.zeros((B_PER_CORE, CST_COLS), np.float32)
    for t in range(9):
        c[:, t] = np.float32(P1C[t + 1])
        c[:, 9 + t] = np.float32(P3C[t + 1])
    c[:, 18] = np.float32(P1C[0])
    c[:, 19] = np.float32(P3C[0])
    eye = np.eye(3, dtype=np.float32).reshape(9)
    c[:, EYE9] = eye
    c[:, EYE3] = eye / 3.0
    c[:, RCL] = RCLAMP
    c[:, TNY] = TINY
    return c


def build_program():
    nc = bacc.Bacc("TRN2", target_bir_lowering=False, debug=False,
                   num_devices=N_CORES)
    P = B_PER_CORE

    cst_d = nc.dram_tensor("cst", [P, CST_COLS], f32, kind="ExternalInput")
    blka_d = nc.dram_tensor("blka", [P, BLKA_COLS], bf16,
                            kind="ExternalInput")
    blkb_d = nc.dram_tensor("blkb", [P, BLKB_COLS], bf16,
                            kind="ExternalInput")
    vab_d = nc.dram_tensor("vab", [128, 2 * F_PACK], bf16,
                           kind="ExternalInput")
    out_d = nc.dram_tensor("out", [128, 8], f32, kind="ExternalOutput")

    with tile.TileContext(nc) as tc, ExitStack() as ctx:
        V = nc.vector
        A = nc.scalar
        G = nc.gpsimd
        SP = nc.sync
        sg = ctx.enter_context(tc.tile_pool(name="singles", bufs=1))
        vp = ctx.enter_context(tc.tile_pool(name="vp", bufs=2))

        def S(shape, name, dtype=f32):
            return sg.tile(list(shape), dtype, name=name)

        comp = S([128, 8], "comp")

        # first ACT op is a Sqrt so the table loader picks the sqrt set once
        warm = S([1, 1], "warm")
        G.memset(warm[:, :], 1.0)
        warm2 = S([1, 1], "warm2")
        A.activation(warm2[:, :], warm[:, :], AF.Sqrt)

        # ---------------- input DMAs ----------------------------------------
        blk_t = S([P, BLKA_COLS], "blk_t", bf16)
        A.dma_start(blk_t[:, :], blka_d[:, :])
        cst_t = S([P, CST_COLS], "cst_t")
        A.dma_start(cst_t[:, :], cst_d[:, :])
        blkb_t = S([P, BLKB_COLS], "blkb_t", bf16)
        A.dma_start(blkb_t[:, :], blkb_d[:, :])
        vab_ts = []
        off = 0
        for c, w in enumerate(W_CHUNKS):
            vt = sg.tile([128, 2 * w], bf16, name=f"vab{c}")
            SP.dma_start(vt[:, :], vab_d[:, 2 * off:2 * off + 2 * w])
            vab_ts.append(vt)
            off += w

        pg6 = blk_t[:, PG6]
        eye9 = cst_t[:, EYE9]
        eye3 = cst_t[:, EYE3]
        t1 = S([P, 1], "t1")

        # ================ Procrustes chain (DVE) ================
        musum = S([P, 6], "musum")
        V.tensor_reduce(musum[:, :], pg6.rearrange("p (c n) -> p c n", n=J),
                        axis=AX.X, op=OP.add)
        Xn = S([P, 144], "Xn", bf16)   # (musum/24 - pg6): negated centered
        V.scalar_tensor_tensor(
            Xn[:, :].rearrange("p (c n) -> p c n", n=J),
            musum[:, :].unsqueeze(2).broadcast_to([P, 6, J]), 1.0 / J,
            pg6.rearrange("p (c n) -> p c n", n=J), OP.mult, OP.subtract)
        X1n = Xn[:, 0:72]
        X2n = Xn[:, 72:144]
        var1 = S([P, 1], "var1")
        vscr = S([P, 72], "vscr")
        A.activation(vscr[:, :], X1n, AF.Square, accum_out=var1[:, :])
        G.tensor_scalar(t1[:, :], blk_t[:, CAM][:, 0:1], 512.0, EPS,
                        OP.mult, OP.add)
        rt1 = S([P, 1], "rt1")
        V.reciprocal(rt1[:, :], t1[:, :])

        # kp2d prep front-loaded on Pool so rzt slots into the chain early
        depth = S([P, 1], "depth")
        G.tensor_single_scalar(depth[:, :], rt1[:, :], 2000.0, OP.mult)
        pxy = S([P, 48], "pxy", bf16)
        G.tensor_add(pxy[:, :].rearrange("p (c n) -> p c n", n=J),
                     blk_t[:, PG6].rearrange("p (c n) -> p c n", n=J)[:, 0:2],
                     blk_t[:, CAM][:, 1:3].unsqueeze(2).broadcast_to([P, 2, J]))
        pzt = S([P, J], "pzt")
        G.tensor_add(pzt[:, :], blk_t[:, 48:72],
                     depth[:, :].broadcast_to([P, J]))

        # K = X1 X2^T
        kq = S([P, 216], "kq", bf16)
        V.tensor_mul(
            kq[:, :].rearrange("p (i j n) -> p i j n", i=3, j=3),
            X1n.rearrange("p (i n) -> p i n", i=3)
                .unsqueeze(2).broadcast_to([P, 3, 3, J]),
            X2n.rearrange("p (j n) -> p j n", j=3)
                .unsqueeze(1).broadcast_to([P, 3, 3, J]))
        K9 = S([P, 9], "K9")
        i_K9 = V.tensor_reduce(K9[:, :], kq[:, :].rearrange(
            "p (i j n) -> p i j n", i=3, j=3), axis=AX.X, op=OP.add)

        # det(K) on DVE (feeds detA for r, and the sign)
        dQ = S([P, 9], "dQ")
        V.tensor_mul(
            dQ[:, :].rearrange("p (a b) -> p a b", a=3),
            K9[:, 3:6].unsqueeze(2).broadcast_to([P, 3, 3]),
            K9[:, 6:9].unsqueeze(1).broadcast_to([P, 3, 3]))
        dD = S([P, 9], "dD")
        V.tensor_sub(dD[:, :].rearrange("p (a b) -> p a b", a=3),
                     dQ[:, :].rearrange("p (a b) -> p a b", a=3),
                     dQ[:, :].rearrange("p (b a) -> p a b", b=3))
        du1 = S([P, 2], "du1")
        i_du1 = G.tensor_mul(du1[:, :], K9[:, 0:2], dD[:, 5:7])
        du2 = S([P, 1], "du2")
        G.tensor_mul(du2[:, :], K9[:, 2:3], dD[:, 1:2])
        du1r = S([P, 1], "du1r")
        G.tensor_add(du1r[:, :], du1[:, 0:1], du1[:, 1:2])
        detK = S([P, 1], "detK")
        G.tensor_add(detK[:, :], du1r[:, :], du2[:, :])
        detA = S([P, 1], "detA")
        G.tensor_mul(detA[:, :], detK[:, :], detK[:, :])
        sg0 = S([P, 1], "sg0")
        G.tensor_single_scalar(sg0[:, :], detK[:, :], 0.0, OP.is_ge)
        sgn = S([P, 1], "sgn")
        i_sgn = G.tensor_scalar(sgn[:, :], sg0[:, :], 2.0, -1.0,
                                OP.mult, OP.add)

        # A = K^T K
        aq = S([P, 27], "aq")  # keep f32: A feeds the eigen chain
        V.tensor_mul(
            aq[:, :].rearrange("p (i j k) -> p i j k", i=3, j=3),
            K9[:, :].rearrange("p (k i) -> p i k", k=3)
                .unsqueeze(2).broadcast_to([P, 3, 3, 3]),
            K9[:, :].rearrange("p (k j) -> p j k", k=3)
                .unsqueeze(1).broadcast_to([P, 3, 3, 3]))
        A9 = S([P, 9], "A9")
        V.tensor_reduce(A9[:, :], aq[:, :].rearrange(
            "p (i j k) -> p i j k", i=3, j=3), axis=AX.X, op=OP.add)
        # A^2 on Pool (mul + 2 strided adds): needed only at W
        a2q = S([P, 27], "a2q")
        G.tensor_mul(
            a2q[:, :].rearrange("p (i j k) -> p i j k", i=3, j=3),
            A9[:, :].rearrange("p (i k) -> p i k", i=3)
                .unsqueeze(2).broadcast_to([P, 3, 3, 3]),
            A9[:, :].rearrange("p (k j) -> p j k", k=3)
                .unsqueeze(1).broadcast_to([P, 3, 3, 3]))
        a2p = S([P, 9], "a2p")
        G.tensor_add(a2p[:, :],
                     a2q[:, :].rearrange("p (m k) -> p m k", k=3)[:, :, 0],
                     a2q[:, :].rearrange("p (m k) -> p m k", k=3)[:, :, 1])
        A29 = S([P, 9], "A29")
        G.tensor_add(A29[:, :], a2p[:, :],
                     a2q[:, :].rearrange("p (m k) -> p m k", k=3)[:, :, 2])

        qsum = S([P, 1], "qsum")
        V.tensor_reduce(qsum[:, :], A9[:, 0:9:4], axis=AX.X, op=OP.add)
        q3rd = S([P, 1], "q3rd")
        V.tensor_single_scalar(q3rd[:, :], qsum[:, :], 1.0 / 3.0, OP.mult)
        q2 = S([P, 1], "q2")
        V.tensor_mul(q2[:, :], q3rd[:, :], q3rd[:, :])
        nqsum = S([P, 1], "nqsum")
        V.tensor_single_scalar(nqsum[:, :], qsum[:, :], -1.0, OP.mult)
        aqn = S([P, 9], "aqn")
        V.scalar_tensor_tensor(aqn[:, :], eye3, qsum[:, :], A9[:, :],
                               OP.mult, OP.subtract)
        pscr = S([P, 9], "pscr")
        V.tensor_mul(pscr[:, :], aqn[:, :], aqn[:, :])
        p2r = S([P, 1], "p2r")
        V.tensor_reduce(p2r[:, :], pscr[:, :], axis=AX.X, op=OP.add)
        p2g = S([P, 1], "p2g")
        i_p2g = V.tensor_scalar(p2g[:, :], p2r[:, :], 1.0 / 6.0, TINY,
                                OP.mult, OP.max)
        pp = S([P, 1], "pp")
        i_pp = A.activation(pp[:, :], p2g[:, :], AF.Sqrt)
        tp = S([P, 1], "tp")
        V.tensor_single_scalar(tp[:, :], pp[:, :], 2.0, OP.mult)

        # z = detA + q*(3 p^2 - q^2)  (fills the sqrt wait)
        zq = S([P, 1], "zq")
        V.scalar_tensor_tensor(zq[:, :], p2g[:, :], 3.0, q2[:, :],
                               OP.mult, OP.subtract)
        zv = S([P, 1], "zv")
        V.tensor_mul(zv[:, :], q3rd[:, :], zq[:, :])
        zz = S([P, 1], "zz")
        V.tensor_add(zz[:, :], detA[:, :], zv[:, :])

        # ---------------- kp3d (Pool prep, ACT accumulate) ------------------
        pd = S([P, 72], "pd", bf16)
        i_pd = G.tensor_sub(pd[:, :], blk_t[:, 0:72], blk_t[:, 72:144])
        pdr = pd[:, :].rearrange("p (c n) -> p c n", n=J)
        pel = S([P, 3], "pel", bf16)
        G.tensor_add(pel[:, :], pdr[:, :, 2].squeeze(), pdr[:, :, 3].squeeze())
        pel2 = S([P, 3], "pel2", bf16)
        G.tensor_single_scalar(pel2[:, :], pel[:, :], 0.5, OP.mult)
        d3n = S([P, 72], "d3n", bf16)
        G.tensor_sub(d3n[:, :].rearrange("p (c n) -> p c n", n=J),
                     pdr, pel2[:, :].unsqueeze(2).broadcast_to([P, 3, J]))
        u3d = S([P, 72], "u3d", bf16)
        G.tensor_mul(u3d[:, :].rearrange("p (c n) -> p c n", n=J),
                     d3n[:, :].rearrange("p (c n) -> p c n", n=J),
                     blk_t[:, CONF3].unsqueeze(1).broadcast_to([P, 3, J]))
        kscr3 = S([P, 72], "kscr3")
        A.activation(kscr3[:, :], u3d[:, :], AF.Abs,
                     accum_out=comp[0:P, 1:2])

        # pose/betas subs (Pool) + Square-accumulate (ACT); host pre-masked
        dp = S([P, 216], "dp", bf16)
        i_dp = G.tensor_sub(dp[:, :], blkb_t[:, RP], blkb_t[:, RG])
        pscr2 = S([P, 216], "pscr2", bf16)
        i_pscr2 = A.activation(pscr2[:, :], dp[:, :], AF.Square,
                               accum_out=comp[0:P, 3:4])
        db = S([P, 10], "db", bf16)
        G.tensor_sub(db[:, :], blkb_t[:, PB], blkb_t[:, GS])
        bscr = S([P, 10], "bscr", bf16)
        A.activation(bscr[:, :], db[:, :], AF.Square,
                     accum_out=comp[0:P, 4:5])

        # r = clamp(z/(2 p p^2)) via a single reciprocal
        up = S([P, 1], "up")
        V.tensor_mul(up[:, :], tp[:, :], p2g[:, :])
        ru = S([P, 1], "ru")
        V.reciprocal(ru[:, :], up[:, :])
        r1 = S([P, 1], "r1")
        i_r1 = V.scalar_tensor_tensor(r1[:, :], zz[:, :], ru[:, :],
                                      cst_t[:, RCL], OP.mult, OP.min)
        # pinv/pv2 feed only the scale coefficient: off the critical path
        pinv = S([P, 1], "pinv")
        i_pinv = V.reciprocal(pinv[:, :], pp[:, :])
        pv2 = S([P, 1], "pv2")
        V.tensor_mul(pv2[:, :], pinv[:, :], pinv[:, :])

        # powers of r: pw = [r, r^2, ..., r^9]
        pw = S([P, 9], "pw")
        V.tensor_single_scalar(pw[:, 0:1], r1[:, :], -RCLAMP, OP.max)
        V.tensor_mul(pw[:, 1:2], pw[:, 0:1], pw[:, 0:1])
        V.tensor_scalar_mul(pw[:, 2:4], pw[:, 0:2], pw[:, 1:2])
        V.tensor_scalar_mul(pw[:, 4:8], pw[:, 0:4], pw[:, 3:4])
        V.tensor_mul(pw[:, 8:9], pw[:, 3:4], pw[:, 4:5])
        # both outer-root polynomials from one mul + one reduce
        pprod = S([P, 18], "pprod")
        V.tensor_mul(pprod[:, :].rearrange("p (g t) -> p g t", g=2),
                     cst_t[:, PC].rearrange("p (g t) -> p g t", g=2),
                     pw[:, :].unsqueeze(1).broadcast_to([P, 2, 9]))
        xr = S([P, 2], "xr")
        V.tensor_reduce(xr[:, :], pprod[:, :].rearrange(
            "p (g t) -> p g t", g=2), axis=AX.X, op=OP.add)
        x = S([P, 2], "xroots")
        V.tensor_add(x[:, :], xr[:, :], cst_t[:, C0])

        # rzt here: Pool's pzt is ready by now, so DVE never stalls on it
        rzt = S([P, J], "rzt")
        i_rzt = V.reciprocal(rzt[:, :], pzt[:, :])
        aa = S([P, 48], "aa")
        G.tensor_mul(aa[:, :].rearrange("p (c n) -> p c n", n=J),
                     pxy[:, :].rearrange("p (c n) -> p c n", n=J),
                     rzt[:, :].unsqueeze(1).broadcast_to([P, 2, J]))
        dkp = S([P, 48], "dkp")
        G.tensor_sub(dkp[:, :], aa[:, :], blkb_t[:, G2])
        u2d = S([P, 48], "u2d")
        G.tensor_mul(u2d[:, :].rearrange("p (c n) -> p c n", n=J),
                     dkp[:, :].rearrange("p (c n) -> p c n", n=J),
                     blkb_t[:, CONF2].unsqueeze(1).broadcast_to([P, 2, J]))
        kscr = S([P, 48], "kscr")
        A.activation(kscr[:, :], u2d[:, :], AF.Abs,
                     accum_out=comp[0:P, 0:1])

        # eigenvalues: lam = [l1, lmid, l3=detA/(l1*lmid)], clamped >= TINY
        lamt = S([P, 3], "lamt")
        V.scalar_tensor_tensor(lamt[:, 0:3:2], x[:, :], tp[:, :],
                               q3rd[:, :].broadcast_to([P, 2]),
                               OP.mult, OP.add)
        t13 = S([P, 1], "t13")
        V.tensor_add(t13[:, :], lamt[:, 0:1], lamt[:, 2:3])
        V.tensor_sub(lamt[:, 1:2], qsum[:, :], t13[:, :])
        t12g = S([P, 1], "t12g")
        V.scalar_tensor_tensor(t12g[:, :], lamt[:, 0:1], lamt[:, 1:2],
                               cst_t[:, TNY], OP.mult, OP.max)
        rt12 = S([P, 1], "rt12")
        V.reciprocal(rt12[:, :], t12g[:, :])
        V.tensor_mul(lamt[:, 2:3], detA[:, :], rt12[:, :])
        lam = S([P, 3], "lam")
        i_lam = V.tensor_single_scalar(lam[:, :], lamt[:, :], TINY, OP.max)
        s3t = S([P, 3], "s3t")
        i_s3t = A.activation(s3t[:, :], lam[:, :], AF.Sqrt)

        # fill the sqrt wait: gap products + scale coefficient
        v1i = S([P, 1], "v1i")
        V.reciprocal(v1i[:, :], var1[:, :])
        cpre = S([P, 1], "cpre")   # pv2 * v1i / 3
        V.scalar_tensor_tensor(cpre[:, :], pv2[:, :], 1.0 / 3.0,
                               v1i[:, :], OP.mult, OP.mult)
        gA = S([P, 2], "gA")   # [l1-lmid, lmid-l3]
        V.tensor_sub(gA[:, :], lam[:, 0:2], lam[:, 1:3])
        g02 = S([P, 1], "g02")
        V.tensor_add(g02[:, :], gA[:, 0:1], gA[:, 1:2])
        Dt = S([P, 3], "Dt")   # signed gap products
        V.tensor_mul(Dt[:, 0:1], gA[:, 0:1], g02[:, :])
        V.scalar_tensor_tensor(Dt[:, 1:2], gA[:, 0:1], -1.0, gA[:, 1:2],
                               OP.mult, OP.mult)
        V.scalar_tensor_tensor(Dt[:, 2:3], g02[:, :], sgn[:, :],
                               gA[:, 1:2], OP.mult, OP.mult)
        rD = S([P, 3], "rD")
        V.reciprocal(rD[:, :], Dt[:, :])

        # scl = (s1+s2+sgn*s3) * cpre  (post-sqrt: 4 small ops)
        sinv = S([P, 3], "sinv")
        V.reciprocal(sinv[:, :], s3t[:, :])
        s0s2 = S([P, 1], "s0s2")
        V.scalar_tensor_tensor(s0s2[:, :], s3t[:, 2:3], sgn[:, :],
                               s3t[:, 0:1], OP.mult, OP.add)
        ssum = S([P, 1], "ssum")
        V.tensor_add(ssum[:, :], s0s2[:, :], s3t[:, 1:2])
        scl = S([P, 1], "scl")
        V.tensor_mul(scl[:, :], ssum[:, :], cpre[:, :])

        # mm9: [m | m*lam | m*linv] -> one reduce gives (al2, t1, t0)
        linv = S([P, 3], "linv")
        V.tensor_mul(linv[:, :], sinv[:, :], sinv[:, :])
        mm9 = S([P, 9], "mm9")
        V.tensor_mul(mm9[:, 0:3], rD[:, :], sinv[:, :])
        V.tensor_mul(mm9[:, 3:6], mm9[:, 0:3], lam[:, :])
        V.tensor_mul(mm9[:, 6:9], mm9[:, 0:3], linv[:, :])
        asum = S([P, 3], "asum")
        i_asum = V.tensor_reduce(asum[:, :], mm9[:, :].rearrange(
            "p (g i) -> p g i", g=3), axis=AX.X, op=OP.add)
        al1 = S([P, 1], "al1")
        V.scalar_tensor_tensor(al1[:, :], asum[:, 0:1], nqsum[:, :],
                               asum[:, 1:2], OP.mult, OP.add)
        al0 = S([P, 1], "al0")
        V.tensor_mul(al0[:, :], asum[:, 2:3], detA[:, :])

        aI = S([P, 9], "aI")
        V.tensor_scalar_mul(aI[:, :], eye9, al0[:, :])
        W1 = S([P, 9], "W1")
        V.scalar_tensor_tensor(W1[:, :], A29[:, :], asum[:, 0:1], aI[:, :],
                               OP.mult, OP.add)
        W9 = S([P, 9], "W9")
        V.scalar_tensor_tensor(W9[:, :], A9[:, :], al1[:, :], W1[:, :],
                               OP.mult, OP.add)

        # R = W K^T ; RX1 ; Y ; d2
        rq = S([P, 27], "rq")
        V.tensor_mul(
            rq[:, :].rearrange("p (a b c) -> p a b c", a=3, b=3),
            W9[:, :].rearrange("p (a c) -> p a c", a=3)
                .unsqueeze(2).broadcast_to([P, 3, 3, 3]),
            K9[:, :].rearrange("p (b c) -> p b c", b=3)
                .unsqueeze(1).broadcast_to([P, 3, 3, 3]))
        R9b = S([P, 9], "R9b", bf16)
        with nc.allow_low_precision(reason="R entries; 3-term reduce"):
            V.tensor_reduce(R9b[:, :], rq[:, :].rearrange(
                "p (a b c) -> p a b c", a=3, b=3), axis=AX.X, op=OP.add)
        rxq = S([P, 216], "rxq", bf16)
        V.tensor_mul(
            rxq[:, :].rearrange("p (i n j) -> p i n j", i=3, n=J),
            R9b[:, :].rearrange("p (i j) -> p i j", i=3)
                .unsqueeze(2).broadcast_to([P, 3, J, 3]),
            X1n.rearrange("p (j n) -> p n j", j=3)
                .unsqueeze(1).broadcast_to([P, 3, J, 3]))
        rx1 = S([P, 72], "rx1")
        V.tensor_reduce(rx1[:, :].rearrange("p (i n) -> p i n", i=3),
                        rxq[:, :].rearrange("p (i n j) -> p i n j",
                                            i=3, n=J),
                        axis=AX.X, op=OP.add)
        Yt = S([P, 72], "Yt")
        V.scalar_tensor_tensor(Yt[:, :], rx1[:, :], scl[:, :], X2n,
                               OP.mult, OP.subtract)
        Y2 = S([P, 72], "Y2")
        V.tensor_mul(Y2[:, :], Yt[:, :], Yt[:, :])
        d2 = S([P, J], "d2")
        V.tensor_reduce(d2[:, :],
                        Y2[:, :].rearrange("p (i n) -> p n i", i=3),
                        axis=AX.X, op=OP.add)
        dscr = S([P, J], "dscr")
        A.activation(dscr[:, :], d2[:, :], AF.Sqrt,
                     accum_out=comp[0:P, 5:6])

        # ---------------- vertex L1 (DVE add + ACT Abs-accumulate) ----------
        vacc = S([128, len(W_CHUNKS)], "vacc")
        i_adds, i_abss = [], []
        for c, w in enumerate(W_CHUNKS):
            vt = vab_ts[c]
            d_t = vp.tile([128, w], bf16, name=f"d{c}", tag="d")
            i_adds.append(V.tensor_add(d_t[:, :], vt[:, 0:w],
                                       vt[:, w:2 * w]))
            s_t = vp.tile([128, w], bf16, name=f"s{c}", tag="s")
            i_abss.append(A.activation(s_t[:, :], d_t[:, :], AF.Abs,
                                       accum_out=vacc[:, c:c + 1]))
        V.tensor_reduce(comp[:, 2:3], vacc[:, :], axis=AX.X, op=OP.add)

        # schedule pins (add_dep_helper(a, b) == a waits on b): keep the
        # vertex adds in the chain's ACT-wait windows, the sqrts ahead of
        # the long Abs ops on ACT, and the Pool det-branch ahead of preps
        for dep, on, why in [
            (i_adds[0], i_r1, "add0 after the r clamp"),
            (i_adds[1], i_r1, "add1 after the r clamp"),
            (i_adds[2], i_lam, "add2 fills the s-sqrt wait"),
            (i_adds[3], i_asum, "add3 late in the chain"),
            (i_rzt, i_p2g, "rzt off the pre-sqrt region"),
            (i_abss[0], i_pp, "p sqrt before the long abs0"),
            (i_abss[1], i_pp, "p sqrt before the long abs1"),
            (i_abss[2], i_s3t, "s sqrt before the long abs2"),
            (i_pd, i_sgn, "Pool det-branch before kp3d prep"),
            (i_pscr2, i_pp, "p-sqrt before the pose square on ACT"),
            (i_pinv, i_r1, "scale-coefficient recip off the r path"),
            (i_dp, i_sgn, "Pool det-branch before pose prep"),
        ]:
            tile.add_dep_helper(dep.ins, on.ins, sync=False, reason=why)

        # ---------------- output (SP queue, split) --------------------------
        SP.dma_start(out_d[:, 0:5], comp[:, 0:5])
        SP.dma_start(out_d[:, 5:8], comp[:, 5:8])

    nc.compile()
    return nc


_PROGRAM = None


def _get_program():
    global _PROGRAM
    if _PROGRAM is None:
        _PROGRAM = build_program()
    return _PROGRAM


def make_in_maps(inputs: dict) -> list:
    import ml_dtypes

    pj = np.asarray(inputs["pred_joints"], np.float32)
    cam = np.asarray(inputs["pred_camera"], np.float32)
    g2 = np.asarray(inputs["gt_keypoints_2d"], np.float32)
    g3 = np.asarray(inputs["gt_keypoints_3d"], np.float32)
    rp = np.asarray(inputs["pred_rotmat"], np.float32).reshape(512, 216)
    rg = np.asarray(inputs["gt_rotmat"], np.float32).reshape(512, 216)
    pb = np.asarray(inputs["pred_betas"], np.float32)
    gs = np.asarray(inputs["gt_shape"], np.float32)
    hs = np.asarray(inputs["has_smpl"], np.int32)
    va = np.asarray(inputs["pred_vertices"], np.float32).reshape(512, VERT_F)
    vb = np.asarray(inputs["gt_vertices"], np.float32).reshape(512, VERT_F)
    cst = _cst_array()

    idx = np.nonzero(hs > 0)[0]
    assert idx.size <= N_CORES * PACK_CAP, (
        f"n_valid={idx.size} exceeds vertex pack capacity")

    def packed(src, sel, negate):
        buf = np.zeros(128 * F_PACK, ml_dtypes.bfloat16)
        if sel.size:
            flat = src[sel].reshape(-1)
            if negate:
                flat = -flat
            buf[:flat.size] = flat.astype(ml_dtypes.bfloat16)
        return buf.reshape(128, F_PACK)

    in_maps = []
    for c in range(N_CORES):
        sl = slice(B_PER_CORE * c, B_PER_CORE * (c + 1))
        sel = idx[c::N_CORES]
        mask = (hs[sl] > 0).astype(np.float32)[:, None]
        blka = np.empty((B_PER_CORE, BLKA_COLS), np.float32)
        blka[:, 0:72] = pj[sl].transpose(0, 2, 1).reshape(B_PER_CORE, 72)
        blka[:, 72:144] = g3[sl, :, :3].transpose(0, 2, 1).reshape(
            B_PER_CORE, 72)
        blka[:, CONF3] = g3[sl, :, 3]
        blka[:, CAM] = cam[sl]
        blkb = np.empty((B_PER_CORE, BLKB_COLS), np.float32)
        blkb[:, G2] = ((g2[sl, :, :2] - 256.0) / 1000.0).transpose(
            0, 2, 1).reshape(B_PER_CORE, 48)
        blkb[:, CONF2] = g2[sl, :, 2] * 1000.0
        blkb[:, RP] = rp[sl] * mask
        blkb[:, RG] = rg[sl] * mask
        blkb[:, PB] = pb[sl] * mask
        blkb[:, GS] = gs[sl] * mask
        va_p = packed(va, sel, False)
        vb_p = packed(vb, sel, True)
        vab = np.empty((128, 2 * F_PACK), ml_dtypes.bfloat16)
        off = 0
        for w in W_CHUNKS:
            vab[:, 2 * off:2 * off + w] = va_p[:, off:off + w]
            vab[:, 2 * off + w:2 * off + 2 * w] = vb_p[:, off:off + w]
            off += w
        in_maps.append({
            "cst": np.ascontiguousarray(cst, np.float32),
            "blka": np.ascontiguousarray(blka.astype(ml_dtypes.bfloat16)),
            "blkb": np.ascontiguousarray(blkb.astype(ml_dtypes.bfloat16)),
            "vab": np.ascontiguousarray(vab),
        })
    return in_maps


def combine_partials(parts: np.ndarray, n_valid: float) -> np.float32:
    # parts: [n_cores, 128, 8]
    p64 = parts.astype(np.float64)
    kp2d = p64[:, 0:B_PER_CORE, 0].sum()
    kp3d = p64[:, 0:B_PER_CORE, 1].sum()
    vert = p64[:, :, 2].sum()
    pose = p64[:, 0:B_PER_CORE, 3].sum()
    betas = p64[:, 0:B_PER_CORE, 4].sum()
    pa = p64[:, 0:B_PER_CORE, 5].sum()
    B = 512.0
    total = (4.0 * kp2d / (512.0 * B * J * 2)
             + 4.0 * kp3d / (B * J * 3)
             + vert / (n_valid * VERT_F + EPS)
             + pose / (n_valid * 216 + EPS)
             + 0.01 * betas / (n_valid * 10 + EPS)
             + pa / (B * J))
    return np.float32(total)


def kernel(**inputs) -> np.ndarray:
    nc = _get_program()
    in_maps = make_in_maps(inputs)
    res = run_bass_kernel_spmd(nc, in_maps, core_ids=list(range(N_CORES)))
    parts = np.stack([res.results[c]["out"] for c in range(N_CORES)])
    nv = float((np.asarray(inputs["has_smpl"]) > 0).sum())
    return np.asarray(combine_partials(parts, nv))
